# revision 14
# baseline (speedup 1.0000x reference)
"""3-layer GCN (PyG GCNConv semantics) on 8 trn2 NeuronCores.

Design notes (v2 — hardware-loop edition):
- On this platform, every *static* instruction costs ~40-65us of dispatch
  (globally serialized across engines), while iterations of a `For_i`
  hardware loop cost ~1.5-5us. The kernel therefore keeps the static
  program tiny and runs all hot work inside For_i loops.
- Math: with A_hat = D^-1/2 (A+I) D^-1/2, each layer is
  x' = LeakyReLU(A_hat @ (x W^T) + b). By associativity we aggregate
  FIRST and multiply by W after: seg = sum_{src in N(dst)} u[src],
  x' = phi(dis * (seg @ W^T)), with the table u = dis*x pre-scaled so no
  per-edge work remains. LeakyReLU's positive homogeneity moves the dis
  factors into two per-column broadcast multiplies.
- Per layer: bf16 AllGather of the u-table -> transpose-mode dma_gather
  of K fixed slots per destination (padded with pointers to an
  all-zero table row) giving d-major messages -> one DVE tensor_reduce
  per gather call (segment sum) -> 14 wt-stationary matmuls (seg slices
  as the 448-wide moving operand) -> LeakyReLU + dis-scale (2 static DVE
  ops) -> 49 xbar DMA-transposes (SBUF->SBUF) back to node-major -> DMA
  to DRAM as next layer's AllGather input.
- int16 gather indices limit a table to 32767 rows, so the 50176-row
  table is split in lo/hi halves of 25088 rows; each dst has K_LO lo
  slots and K_HI hi slots, reduced separately and summed for free in
  PSUM (two accumulating matmuls).
"""
import numpy as np
import ml_dtypes

from concourse import bacc, tile, mybir, bass
from concourse.bass import ds, ts
from concourse.bass_utils import run_bass_kernel_spmd

N, D, L, NCORES = 50000, 128, 3, 8
PER = N // NCORES            # 6250 nodes per core
CHUNKS = 49                  # 49*128 = 6272 padded local rows
NLP = CHUNKS * 128
HALF = (NCORES // 2) * NLP   # 25088 rows per table half
LEAKY = 0.01
NDG = 64                     # destinations per gather call
NGRP = NLP // NDG            # 98 groups (2x1280 descs in flight fits ring)
REPEATS = 1

f32 = mybir.dt.float32
bf16 = mybir.dt.bfloat16
i16 = mybir.dt.int16


# ---------------------------------------------------------------- host prep

def _prep(edge_index):
    src = np.concatenate([edge_index[0], np.arange(N)]).astype(np.int64)
    dst = np.concatenate([edge_index[1], np.arange(N)]).astype(np.int64)
    deg = np.bincount(dst, minlength=N)
    dis = (1.0 / np.sqrt(np.maximum(deg, 1))).astype(np.float32)

    core_of = dst // PER
    li_dst = dst - core_of * PER          # local column 0..6249
    islo = src < (N // 2)

    dlo = np.bincount(dst[islo], minlength=N)
    dhi = np.bincount(dst[~islo], minlength=N)
    # K must make NDG*K a multiple of 128 (NDG=224 -> K % 4 == 0)
    KMAX = max(int(dlo.max()), int(dhi.max()))
    K_LO = K_HI = int(-(-KMAX // 4) * 4)
    SL = NLP * K_LO
    SH = NLP * K_HI

    # trow: table row of each node (chunk-padded, id order within core)
    trow = (np.arange(N) // PER) * NLP + (np.arange(N) % PER)

    ZROW = PER  # local row 6250 of core 0 (lo) / core 4 (hi) is all-zero

    gidx = np.full((NCORES, SL + SH), ZROW, np.int16)
    for hi in (False, True):
        m = ~islo if hi else islo
        d_e = dst[m]
        s_e = src[m]
        order = np.argsort(d_e, kind="stable")
        d_e = d_e[order]
        s_e = s_e[order]
        cnt = np.bincount(d_e, minlength=N)
        starts = np.concatenate([[0], np.cumsum(cnt)[:-1]])
        k = np.arange(len(d_e)) - np.repeat(starts[cnt > 0], cnt[cnt > 0])
        K = K_HI if hi else K_LO
        base = SL if hi else 0
        li = d_e % PER
        slot = base + li * K + k
        val = trow[s_e] - (HALF if hi else 0)
        gidx[d_e // PER, slot] = val.astype(np.int16)

    # wrap to [128, S/16] int16 (16-partition wrap, replicated x8)
    S = SL + SH
    gidx_w = np.ascontiguousarray(
        np.tile(gidx.reshape(NCORES, S // 16, 16).transpose(0, 2, 1), (1, 8, 1)))

    # per-core dis rows (padded cols = 0)
    disrow1 = np.zeros((NCORES, 1, NLP), np.float32)
    disrow2 = np.zeros((NCORES, 1, NLP), np.float32)
    dd = dis.reshape(NCORES, PER)
    disrow1[:, 0, :PER] = dd
    disrow2[:, 0, :PER] = dd * dd

    return dict(K_LO=K_LO, K_HI=K_HI, gidx_w=gidx_w,
                disrow1=disrow1, disrow2=disrow2, dis=dis)


# ------------------------------------------------------------- device build

def _build(K_LO, K_HI, has_bias):
    SL = NLP * K_LO
    NI_L = NDG * K_LO
    NI_H = NDG * K_HI
    COLS_L = NI_L // 16
    COLS_H = NI_H // 16
    SCOL_L = SL // 16
    SCOLS = (SL + NLP * K_HI) // 16
    MMW = 448                    # moving width per matmul (14 iters)
    NMM = NLP // MMW

    nc = bacc.Bacc("TRN2", target_bir_lowering=False, debug=False,
                   num_devices=NCORES, num_swdge_queues=2,
                   dynamic_dma_scratch_size=32768)
    u0_d = nc.dram_tensor("u0", [NLP, D], bf16, kind="ExternalInput")
    gidx_d = nc.dram_tensor("gidx", [128, SCOLS], i16, kind="ExternalInput")
    wt_d = nc.dram_tensor("wt", [L, D, D], f32, kind="ExternalInput")
    dr1_d = nc.dram_tensor("disrow1", [1, NLP], f32, kind="ExternalInput")
    dr2_d = nc.dram_tensor("disrow2", [1, NLP], f32, kind="ExternalInput")
    if has_bias:
        bt_d = nc.dram_tensor("bT", [128, L], f32, kind="ExternalInput")
    xout = nc.dram_tensor("xout", [NLP, D], f32, kind="ExternalOutput")

    with tile.TileContext(nc) as tc:
        with (
            tc.tile_pool(name="const", bufs=1) as cpool,
            tc.tile_pool(name="ps", bufs=2, space="PSUM") as pspool,
            tc.tile_pool(name="dram", bufs=1, space="DRAM") as dpool,
        ):
            gidx_t = cpool.tile([128, SCOLS], i16)
            wt_t = cpool.tile([128, L * D], f32)
            disbc1 = cpool.tile([128, NLP], bf16, name="disbc1")
            disbc2 = cpool.tile([128, NLP], bf16, name="disbc2")
            seg_lo = cpool.tile([128, NLP], f32, name="seg_lo")
            seg_hi = cpool.tile([128, NLP], f32, name="seg_hi")
            hT = cpool.tile([128, NLP], bf16, name="hT")
            uTp = cpool.tile([128, NLP], bf16, name="uTp")
            uT = cpool.tile([128, NLP], bf16, name="uT")
            xb = cpool.tile([128, NLP], bf16, name="xb")
            gtLA = cpool.tile([128, 1, NI_L], bf16, name="gtLA")
            gtHA = gtLA  # shared buffer: WAR dep keeps ONE gather in flight
            warm = cpool.tile([128, 1, 128], bf16, name="warm")
            psH = pspool.tile([128, MMW], f32, name="psH")

            nc.sync.dma_start(gidx_t[:], gidx_d[:])
            for l in range(L):
                nc.sync.dma_start(wt_t[:, l * D:(l + 1) * D], wt_d[l])
            for bc, dr in ((disbc1, dr1_d), (disbc2, dr2_d)):
                ap = dr.ap()
                nc.gpsimd.dma_start(
                    out=bc[:],
                    in_=bass.AP(tensor=ap.tensor, offset=ap.offset,
                                ap=[[0, 128], ap.ap[1]]))
            if has_bias:
                bt_t = cpool.tile([128, L], f32)
                nc.sync.dma_start(bt_t[:], bt_d[:])
                dbb = [cpool.tile([128, NLP], bf16, name=f"dbb{l}")
                       for l in range(L)]
                for l in range(L):
                    if l < L - 1:
                        # u = phi(dis^2*h + dis*b): bias term dis*b
                        nc.vector.tensor_scalar(
                            dbb[l][:], disbc1[:], bt_t[:, l:l + 1], None,
                            mybir.AluOpType.mult)
                    else:
                        # x = phi(dis*h + b): bias term b (broadcast cols)
                        nc.vector.tensor_scalar(
                            dbb[l][:], disbc1[:], 0.0, bt_t[:, l:l + 1],
                            mybir.AluOpType.mult, mybir.AluOpType.add)

            hshs = [[dpool.tile([NLP, D], bf16, name=f"hsh{r}_{l}")
                     for l in range(L)] for r in range(REPEATS)]
            tmps = [[dpool.tile([NCORES * NLP, D], bf16, addr_space="Shared",
                                name=f"tmp{r}_{l}") for l in range(L)]
                    for r in range(REPEATS)]

            # warm-up gather: pays the one-time gpsimd ucode load early
            nc.gpsimd.dma_gather(warm[:], tmps[0][0][0:HALF, :],
                                 gidx_t[:, 0:8], 128, 128, D,
                                 single_packet=False, transpose=True)

            for rep in range(REPEATS):
                # stage u0 -> xb -> hsh0 (chains rep r's AG0 after rep r-1)
                nc.sync.dma_start(
                    xb[:].rearrange("p (t d) -> p t d", d=D),
                    u0_d[:].rearrange("(t p) d -> p t d", p=128))
                nc.sync.dma_start(
                    hshs[rep][0][:].rearrange("(t p) d -> p t d", p=128),
                    xb[:].rearrange("p (t d) -> p t d", d=D))

                for l in range(L):
                    tmp = tmps[rep][l]
                    nc.gpsimd.collective_compute(
                        "AllGather", mybir.AluOpType.bypass,
                        replica_groups=[list(range(NCORES))],
                        ins=[hshs[rep][l][:]], outs=[tmp[:]])
                    tab_lo = tmp[0:HALF, :]
                    tab_hi = tmp[HALF:2 * HALF, :]

                    # gather + segment-sum loop. Single gt buffer per table
                    # half: the WAR dep on the reduce paces the next gather
                    # so at most one call is in flight per SWDGE queue (two
                    # concurrent calls overflow the ~2048-pair desc ring).
                    with tc.For_i(0, NGRP) as g:
                        nc.gpsimd.dma_gather(
                            gtLA[:], tab_lo, gidx_t[:, ts(g, COLS_L)],
                            NI_L, NI_L, D, single_packet=False,
                            transpose=True, queue_num=0)
                        nc.vector.tensor_reduce(
                            seg_lo[:, ds(g * NDG, NDG)],
                            gtLA[:].rearrange("p o (nd k) -> p (o nd) k",
                                              k=K_LO),
                            mybir.AxisListType.X, mybir.AluOpType.add)
                        nc.gpsimd.dma_gather(
                            gtHA[:], tab_hi,
                            gidx_t[:, ds(g * COLS_H + SCOL_L, COLS_H)],
                            NI_H, NI_H, D, single_packet=False,
                            transpose=True, queue_num=1)
                        nc.vector.tensor_reduce(
                            seg_hi[:, ds(g * NDG, NDG)],
                            gtHA[:].rearrange("p o (nd k) -> p (o nd) k",
                                              k=K_HI),
                            mybir.AxisListType.X, mybir.AluOpType.add)

                    # h^T = W @ (seg_lo + seg_hi): wt stationary, seg moving
                    wl = wt_t[:, l * D:(l + 1) * D]
                    with tc.For_i(0, NMM) as c:
                        nc.tensor.matmul(psH[:], wl, seg_lo[:, ts(c, MMW)],
                                         start=True, stop=False)
                        nc.tensor.matmul(psH[:], wl, seg_hi[:, ts(c, MMW)],
                                         start=False, stop=True)
                        nc.scalar.activation(
                            hT[:, ts(c, MMW)], psH[:],
                            mybir.ActivationFunctionType.Copy)

                    # static post: phi then per-dst scale (homogeneity)
                    if has_bias:
                        # t = hT*dis_bc + dis*b ; phi ; (already scaled)
                        nc.vector.tensor_tensor(
                            uTp[:], hT[:],
                            disbc2[:] if l < L - 1 else disbc1[:],
                            mybir.AluOpType.mult)
                        nc.vector.tensor_tensor(
                            uTp[:], uTp[:], dbb[l][:], mybir.AluOpType.add)
                        nc.vector.scalar_tensor_tensor(
                            uT[:], uTp[:], LEAKY, uTp[:],
                            mybir.AluOpType.mult, mybir.AluOpType.max)
                    else:
                        nc.vector.scalar_tensor_tensor(
                            uTp[:], hT[:], LEAKY, hT[:],
                            mybir.AluOpType.mult, mybir.AluOpType.max)
                        nc.vector.tensor_tensor(
                            uT[:], uTp[:],
                            disbc2[:] if l < L - 1 else disbc1[:],
                            mybir.AluOpType.mult)

                    # back to node-major via xbar transpose (SBUF->SBUF)
                    with tc.For_i(0, CHUNKS) as c:
                        nc.sync.dma_start_transpose(xb[:, ts(c, D)],
                                                    uT[:, ts(c, D)])

                    if l < L - 1:
                        nc.sync.dma_start(
                            hshs[rep][l + 1][:].rearrange(
                                "(t p) d -> p t d", p=128),
                            xb[:].rearrange("p (t d) -> p t d", d=D))

            # final output (bf16 -> f32 cast during DMA)
            nc.gpsimd.dma_start(
                xout[:].rearrange("(t p) d -> p t d", p=128),
                xb[:].rearrange("p (t d) -> p t d", d=D))
    nc.compile()
    return nc


_CACHE = {}
LAST_EXEC_NS = None
LAST_TRACE = None


def kernel(x, edge_index, Ws, bs):
    x = np.asarray(x, np.float32)
    ei = np.asarray(edge_index, np.int64)
    Ws = np.asarray(Ws, np.float32)
    bs = np.asarray(bs, np.float32)
    has_bias = bool(np.any(bs != 0.0))

    p = _prep(ei)
    key = (p["K_LO"], p["K_HI"], has_bias, REPEATS)
    if key not in _CACHE:
        _CACHE[key] = _build(p["K_LO"], p["K_HI"], has_bias)
    nc = _CACHE[key]

    dis = p["dis"]
    u = (x * dis[:, None]).astype(ml_dtypes.bfloat16)
    u0 = np.zeros((NCORES, NLP, D), ml_dtypes.bfloat16)
    u0[:, :PER] = u.reshape(NCORES, PER, D)
    wt = np.ascontiguousarray(Ws.transpose(0, 2, 1))

    in_maps = []
    for c in range(NCORES):
        m = dict(u0=u0[c], gidx=p["gidx_w"][c], wt=wt,
                 disrow1=p["disrow1"][c], disrow2=p["disrow2"][c])
        if has_bias:
            m["bT"] = np.ascontiguousarray(bs.T.astype(np.float32))
        in_maps.append(m)
    res = run_bass_kernel_spmd(nc, in_maps, list(range(NCORES)))
    global LAST_EXEC_NS, LAST_TRACE
    LAST_EXEC_NS = res.exec_time_ns
    LAST_TRACE = res.instructions_and_trace

    allout = np.stack([res.results[c]["xout"][:PER] for c in range(NCORES)])
    return np.ascontiguousarray(allout.reshape(N, D))
